# revision 28
# baseline (speedup 1.0000x reference)
"""MoE layer (B=2,T=1024,D=2048,F=768,E=16,K=2) on 8 NeuronCores.

Expert-parallel: 16 experts sorted by routed-token count; the 8 heaviest go
in slot 0 (capacity C0), the 8 lightest in slot 1 (capacity C1 <= C0), one
of each per core. Host computes the router (softmax -> top-2 -> renormalize,
~0.3% of FLOPs), gathers each expert's tokens into fixed-capacity transposed
buffers, and the device kernel runs the sparse SwiGLU FFN in bf16 with f32
PSUM accumulation. The per-token routing weight is applied on the host
during the scatter-add, so no combine-weight tensor ships to the device.

The kernel sits at the ridge point: ~21MB of input at ~400GB/s aggregate
wire (~53us) vs ~68us of PE matmul issue. The schedule:
- tokens staged transposed (xgt [D, C] as [P, KD*C]) so gate/up produce
  hT [F, C] directly in the lhsT layout the down projection wants.
- gate+up weights are host-packed per j-chunk into one [2, P, KD*P] slab
  so each j ships as a single fused 1.05MB DMA (big transfers keep the
  rings at wire speed; the baseline's turnaround problem).
- the ACT HWDGE ring starts ~1.5us after its first trigger, the SP ring
  ~3.4us (absorbed by a tiny warm-up DMA). Bootstrap tiles (e0 j0 gate/up
  as separate 0.5MB slabs, e0 tokens as 4 quarter DMAs) alternate across
  rings in global deadline order so the first real matmul issues ~t+6us.
- e0 down projection runs in TWO full-PSUM passes (f-tiles 0-3, then 4-5)
  with copy-only evictions split ACT/DVE by m-parity; the partial merge
  (ysbA += ysbB, 16 DVE adds) is deferred into e1's gate phase where the
  DVE is otherwise idle, and y(e0) streams out on the SWDGE queue behind
  it. This keeps the down passes PE-dense (the 3-pass staged variant was
  eviction-bound: DVE 86% busy) while the down-weight slab deadlines stay
  loose (dt00/dt01 by pass A, dt02 by pass B).
- e1's full weight set streams on the SP ring during e0 compute; e1 down
  accumulates all 6 f-tiles in PSUM per m-chunk (16 copy-evictions only).
- ~8 N=512 + 10 N=256 garbage matmuls at kernel start warm the PE clock
  (HAM) during the DMA ramp. The kernel-end tail is dominated by the
  fixed NEFF epilogue (~5.4us per-engine semaphore-reset ladder); the
  last y1 chunks ship as small solo DMAs to minimize the drain ahead
  of it.
"""

import numpy as np
from contextlib import ExitStack

import concourse.bass as bass
import concourse.tile as tile
from concourse import mybir
from concourse.bass_utils import run_bass_kernel_spmd

B, T, D, F, E, TOPK = 2, 1024, 2048, 768, 16, 2
NCORES = 8
EPC = E // NCORES  # experts per core (2 slots)
P = 128


def _split_waits(nc, max_waits=1):
    """walrus on this image rejects >1 sync-wait per instruction
    (setupSyncWait: "Too many sync wait commands"); split extras into
    preceding same-engine NoOps."""
    for f in nc.m.functions:
        for b in f.blocks:
            insts = b.instructions
            idx = 0
            while idx < len(insts):
                inst = insts[idx]
                si = getattr(inst, "sync_info", None)
                if si is not None and si.on_wait and len(si.on_wait) > max_waits:
                    waits = list(si.on_wait)
                    extra, keep = waits[:-max_waits], waits[-max_waits:]
                    pos = idx
                    for j in range(0, len(extra), max_waits):
                        chunk = extra[j : j + max_waits]
                        nop = mybir.InstNoOp(name=f"{inst.name}_ws{j}", ins=[], outs=[])
                        nop.engine = inst.engine
                        nop.sync_info = mybir.SyncInfo(on_wait=chunk, on_update=[])
                        insts.insert(pos, nop)
                        pos += 1
                        idx += 1
                    inst.sync_info = mybir.SyncInfo(
                        on_wait=keep, on_update=list(si.on_update)
                    )
                idx += 1


def build_moe(C0, C1):
    """Per-core kernel: slot 0 capacity C0, slot 1 capacity C1 (each %8==0)."""
    assert C0 % 8 == 0 and C1 % 8 == 0 and 128 <= C1 <= C0 <= 512
    CS = (C0, C1)
    KD = D // P  # 16 k-tiles over D
    KF = F // P  # 6 f-chunks over F
    MD = D // P  # 16 m-chunks over D (down proj, yT layout)
    bf16 = mybir.dt.bfloat16
    f32 = mybir.dt.float32

    nc = bass.Bass("TRN2", target_bir_lowering=False, debug=False, num_devices=NCORES)
    # host pre-tiled layouts (>=2KB contiguous per partition per DMA):
    #   xgt{s}[p, k*C + c] = x_gathered[s, k*128+p, c]
    #   wgu[e, j, g, p, k*128+f] = w_g[e, k*128+p, j*128+f]  (g=0 gate, 1 up)
    xgt0 = nc.declare_dram_parameter("xgt0", [P, KD * C0], bf16, isOutput=False)
    xgt1 = nc.declare_dram_parameter("xgt1", [P, KD * C1], bf16, isOutput=False)
    wg = nc.declare_dram_parameter("wg", [EPC, KF, P, KD * P], bf16, isOutput=False)
    wu = nc.declare_dram_parameter("wu", [EPC, KF, P, KD * P], bf16, isOutput=False)
    wd = nc.declare_dram_parameter("wd", [EPC, F, D], bf16, isOutput=False)
    y0 = nc.declare_dram_parameter("y0", [D, C0], bf16, isOutput=True)
    y1 = nc.declare_dram_parameter("y1", [D, C1], bf16, isOutput=True)
    xgts = (xgt0, xgt1)
    ys = (y0, y1)

    with tile.TileContext(nc) as tc, ExitStack() as ctx:
        xp = ctx.enter_context(tc.tile_pool(name="xp", bufs=1))
        wgp = ctx.enter_context(tc.tile_pool(name="wgp", bufs=1))
        # dt tiles single-buffered: e1's down-weight DMA then naturally waits
        # for e0's pass reads to finish -- a free throttle that keeps that
        # traffic out of the oversubscribed early window.
        wdp = ctx.enter_context(tc.tile_pool(name="wdp", bufs=1))
        hp = ctx.enter_context(tc.tile_pool(name="hp", bufs=1))
        sp = ctx.enter_context(tc.tile_pool(name="sp", bufs=2))
        op = ctx.enter_context(tc.tile_pool(name="op", bufs=1))
        # one shared 8-bank PSUM ring.
        pp = ctx.enter_context(tc.tile_pool(name="pp", bufs=8, space="PSUM"))

        # PE warmup: garbage matmuls with no dependencies run during the
        # initial DMA ramp so HAM un-throttles (1.2->2.4GHz) before real work.
        wsb = sp.tile([P, 512], bf16, tag="warm_sb", bufs=1)
        nc.vector.memset(wsb[:], 0)
        for _ in range(8):
            wps = pp.tile([P, 512], f32, tag="ps")
            nc.tensor.matmul(wps[:], wsb[:, :P], wsb[:], start=True, stop=True)
        for _ in range(10):
            wps = pp.tile([P, 256], f32, tag="ps")
            nc.tensor.matmul(wps[:], wsb[:, :P], wsb[:, :256], start=True, stop=True)

        gts = [[None] * KF for _ in range(EPC)]  # (tile, idx) per j
        uts = [[None] * KF for _ in range(EPC)]
        dts = [[None] * 3 for _ in range(EPC)]
        xparts = [None, None]  # e0: 4 quarter tiles; e1: 1 full tile
        XKT = (4, KD)  # k-tiles per token tile

        def trig_w(e, j, eng, w, store, pref):
            t = wgp.tile(
                [P, 1, KD * P], bf16, tag=f"{pref}{e}j{j}", name=f"{pref}{e}j{j}",
                bufs=1,
            )
            eng.dma_start(t[:], w[e, j : j + 1].rearrange("j p c -> p j c"))
            store[e][j] = (t, 0)

        def trig_dt(e, h, eng):
            dt = wdp.tile([P, KF // 3, D], bf16, tag=f"dt{h}")
            eng.dma_start(
                dt[:],
                wd[e].rearrange("(k p) d -> p k d", p=P)[:, bass.ts(h, KF // 3), :],
            )
            dts[e][h] = dt

        def xop(e, k):
            """Column-slice covering token k-tile k of expert e."""
            kt = XKT[e]
            return xparts[e][k // kt][:, bass.ts(k % kt, CS[e])]

        # Two HWDGE rings, each ~190GB/s when both are backlogged (the
        # aggregate is HBM-wire-bound), FIFO per ring.  The gate phase
        # consumes ~265GB/s, so consumption must ALTERNATE rings every
        # half-slab: all gate halves ride the ACT ring, all up halves and
        # tokens the SP ring, each in deadline order -- per-ring demand is
        # then ~134GB/s and the j-loop stays PE-bound after j1.  A tiny
        # warm-up DMA absorbs the SP ring's ~2us-slower first byte.  All
        # triggers are issued up front (the ACT engine finishes its 15
        # before its first silu is due; the sync engine is idle anyway).
        wdm = sp.tile([P, 16], bf16, tag="warm_dma", bufs=1)
        nc.sync.dma_start(wdm[:], xgt0[:, :16])

        # The HWDGE ring backpressures its trigger instructions once ~4
        # transfers are in flight, so an engine that has real work (ACT:
        # silu + evictions) may only queue 4 up front -- the rest are
        # drip-fed from issue points where the ring has drained (after
        # each silu / between pass-A evictions).  The sync engine is idle
        # all kernel, so its ring's triggers can all block harmlessly up
        # front.
        trig_w(0, 0, nc.scalar, wg, gts, "g")
        # e0 tokens: 4 quarter DMAs (separate tiles so each quarter's
        # consumers wait only for its own transfer).
        xparts[0] = []
        for q in range(4):
            xt = xp.tile([P, 4 * C0], bf16, tag=f"xt0q{q}")
            nc.sync.dma_start(xt[:], xgt0[:, bass.ts(q, 4 * C0)])
            xparts[0].append(xt)
        trig_w(0, 0, nc.scalar, wu, uts, "u")
        trig_w(0, 1, nc.scalar, wg, gts, "g")
        trig_w(0, 2, nc.scalar, wg, gts, "g")
        for j in range(1, KF):
            trig_w(0, j, nc.sync, wu, uts, "u")
        trig_dt(0, 1, nc.sync)
        # e1 tokens (one fused DMA) + up halves (two fused transfers)
        # behind on the SP ring.
        xt1 = xp.tile([P, KD * C1], bf16, tag="xt1")
        nc.sync.dma_start(xt1[:], xgt1[:, :])
        xparts[1] = [xt1]

        def trig_w3(e, j0, eng, w, store, pref):
            t = wgp.tile(
                [P, 3, KD * P], bf16, tag=f"{pref}{e}g{j0}", name=f"{pref}{e}g{j0}",
                bufs=1,
            )
            eng.dma_start(t[:], w[e, j0 : j0 + 3].rearrange("j p c -> p j c"))
            for i in range(3):
                store[e][j0 + i] = (t, i)

        trig_w3(1, 0, nc.sync, wu, uts, "u")
        trig_w3(1, 3, nc.sync, wu, uts, "u")

        def trig_w2(e, j0, eng, w, store, pref):
            t = wgp.tile(
                [P, 2, KD * P], bf16, tag=f"{pref}{e}p{j0}", name=f"{pref}{e}p{j0}",
                bufs=1,
            )
            eng.dma_start(t[:], w[e, j0 : j0 + 2].rearrange("j p c -> p j c"))
            for i in range(2):
                store[e][j0 + i] = (t, i)

        # ACT-ring triggers deferred past the 4 upfront slots, in ring
        # (deadline) order; popped one at a time after each silu while the
        # ACT engine has slack.  e1's gate halves ship as three paired
        # transfers, the first popped at e0's last silu and the rest
        # during pass A, where the ACT engine does nothing else (pass A
        # evictions are DVE-only) so the pops can absorb descriptor-pool
        # blocking harmlessly while still arriving ~9us before their
        # e1-gate deadlines.
        act_trigs = (
            [(lambda j=j: trig_w(0, j, nc.scalar, wg, gts, "g")) for j in (3, 4)]
            + [lambda: trig_dt(0, 0, nc.scalar)]
            + [lambda: trig_w(0, 5, nc.scalar, wg, gts, "g")]
            + [lambda: trig_dt(0, 2, nc.scalar)]
            + [None, lambda: trig_w2(1, 0, nc.scalar, wg, gts, "g")]
        )
        e1g_trigs = [
            (lambda j0=j0: trig_w2(1, j0, nc.scalar, wg, gts, "g")) for j0 in (2, 4)
        ]

        def pop_trig(n=1, src=act_trigs):
            for _ in range(n):
                if src:
                    t = src.pop(0)
                    if t is not None:
                        t()

        for e in range(EPC):
            C = CS[e]
            # ---- gate/up + SwiGLU -> hT [F, C] bf16 ----
            ht = hp.tile([P, KF, C], bf16, tag=f"ht{e}")
            for j in range(KF):
                gt, gi = gts[e][j]
                ut, ui = uts[e][j]
                g_ps = pp.tile([P, C], f32, tag="ps")
                u_ps = pp.tile([P, C], f32, tag="ps")
                if e == 0 and j == 0:
                    # bootstrap: interleave the g/u chains by token
                    # quarter so the PE tracks the arrival ramp instead
                    # of stalling per quarter twice.
                    order = [("g", k) for k in range(4)] + [("u", k) for k in range(4)]
                    order = [
                        ("g", 0), ("g", 1), ("g", 2), ("u", 0),
                        ("u", 1), ("g", 3), ("u", 2), ("u", 3),
                    ]
                    for which, q in order:
                        ps, (t, ti) = (
                            (g_ps, (gt, gi)) if which == "g" else (u_ps, (ut, ui))
                        )
                        for k in range(4 * q, 4 * q + 4):
                            nc.tensor.matmul(
                                ps[:],
                                t[:, ti, bass.ts(k, P)],
                                xop(e, k),
                                start=(k == 0),
                                stop=(k == KD - 1),
                            )
                else:
                    for k in range(KD):
                        nc.tensor.matmul(
                            g_ps[:],
                            gt[:, gi, bass.ts(k, P)],
                            xop(e, k),
                            start=(k == 0),
                            stop=(k == KD - 1),
                        )
                    for k in range(KD):
                        nc.tensor.matmul(
                            u_ps[:],
                            ut[:, ui, bass.ts(k, P)],
                            xop(e, k),
                            start=(k == 0),
                            stop=(k == KD - 1),
                        )
                sil = sp.tile([P, C], f32, tag="sil")
                nc.scalar.activation(
                    sil[:], g_ps[:], mybir.ActivationFunctionType.Silu
                )
                if e == 0:
                    pop_trig(2 if j == 0 else 1)
                nc.vector.tensor_mul(ht[:, j, :], sil[:], u_ps[:])
                if e == 1:
                    # deferred e0 partial merge + y0 drain ride the
                    # otherwise-idle DVE / SWDGE queue during e1's gate.
                    if j < 4:
                        m0 = 4 * j
                        nc.vector.tensor_add(
                            ysbA[:, m0 : m0 + 4, :],
                            ysbA[:, m0 : m0 + 4, :],
                            ysbB[:, m0 : m0 + 4, :],
                        )
                        nc.gpsimd.dma_start(
                            ys[0].rearrange("(m p) c -> p m c", p=P)[
                                :, m0 : m0 + 4, :
                            ],
                            ysbA[:, m0 : m0 + 4, :],
                        )

            # ---- down proj: yT[m] = sum_f dwT[f, m] @ hT[f, :] ----
            ydst = ys[e].rearrange("(m p) c -> p m c", p=P)
            if e == 0:
                # two full-PSUM passes (f-tiles 0-3, then 4-5): copy-only
                # evictions split ACT/DVE by m-parity keep the PE dense;
                # the A+B merge is deferred into e1's gate phase.
                ysbA = op.tile([P, MD, C], bf16, tag="ysbA")
                ysbB = op.tile([P, MD, C], bf16, tag="ysbB")
                for m in range(MD):
                    y_ps = pp.tile([P, C], f32, tag="ps")
                    for hi in range(4):
                        nc.tensor.matmul(
                            y_ps[:],
                            dts[0][hi // 2][:, hi % 2, bass.ts(m, P)],
                            ht[:, hi, :],
                            start=(hi == 0),
                            stop=(hi == 3),
                        )
                    # DVE-only evictions: the ACT engine stays free so the
                    # two e1 gate-weight pops can block on descriptor-pool
                    # space without stalling anything.
                    nc.vector.tensor_copy(ysbA[:, m, :], y_ps[:])
                    if m in (2, 6):
                        pop_trig(1, e1g_trigs)
                # e1 down slabs join the queues here; WAR-gated on e0's
                # pass reads (wdp bufs=1), and their ring backlog places
                # them well before e1's down phase.
                trig_dt(1, 0, nc.scalar)
                trig_dt(1, 1, nc.sync)
                trig_dt(1, 2, nc.sync)
                for m in range(MD):
                    y_ps = pp.tile([P, C], f32, tag="ps")
                    for i in range(2):
                        nc.tensor.matmul(
                            y_ps[:],
                            dts[0][2][:, i, bass.ts(m, P)],
                            ht[:, 4 + i, :],
                            start=(i == 0),
                            stop=(i == 1),
                        )
                    ev = nc.vector.tensor_copy if m % 2 else nc.scalar.copy
                    ev(ysbB[:, m, :], y_ps[:])
                    if m % 8 == 3:
                        pop_trig(1, e1g_trigs)
            else:
                # input is all on-chip: m-major, full 6-tile accumulation
                # in PSUM per m-chunk, one eviction per m-chunk (split
                # ACT/DVE by parity).  m-major means each m completes
                # early in the phase, so the output DMAs spread across it
                # instead of bunching at the end; the last two m ship as
                # SOLO transfers so the final post-matmul DMA is one small
                # transfer on an empty ring (the kernel-end barrier waits
                # on its receipt).
                ysb = op.tile([P, MD, C], bf16, tag="ysb1")
                for m in range(MD):
                    y_ps = pp.tile([P, C], f32, tag="ps")
                    for hi in range(6):
                        nc.tensor.matmul(
                            y_ps[:],
                            dts[e][hi // 2][:, hi % 2, bass.ts(m, P)],
                            ht[:, hi, :],
                            start=(hi == 0),
                            stop=(hi == 5),
                        )
                    ev = nc.vector.tensor_copy if m % 2 else nc.scalar.copy
                    ev(ysb[:, m, :], y_ps[:])
                    if m < 14 and m % 2 == 1:
                        yeng = nc.sync if (m // 2) % 2 == 0 else nc.scalar
                        yeng.dma_start(
                            ydst[:, m - 1 : m + 1, :],
                            ysb[:, m - 1 : m + 1, :],
                        )
                    elif m == 14:
                        nc.scalar.dma_start(
                            ydst[:, m : m + 1, :], ysb[:, m : m + 1, :]
                        )
                    elif m == 15:
                        nc.sync.dma_start(
                            ydst[:, m : m + 1, :], ysb[:, m : m + 1, :]
                        )

    _split_waits(nc)
    return nc


_CACHE = {}


def _get_nc(C0, C1):
    if (C0, C1) not in _CACHE:
        _CACHE[(C0, C1)] = build_moe(C0, C1)
    return _CACHE[(C0, C1)]


def _route(x, router_w):
    """Replicates the reference router in f32: softmax over expert scores,
    top-2, renormalize."""
    xf = x.reshape(-1, D).astype(np.float32)
    scores = xf @ router_w.astype(np.float32)
    m = scores.max(axis=-1, keepdims=True)
    ex = np.exp(scores - m)
    probs = ex / ex.sum(axis=-1, keepdims=True)
    idx = np.argsort(-probs, axis=-1, kind="stable")[:, :TOPK]
    wts = np.take_along_axis(probs, idx, axis=-1)
    wts = wts / wts.sum(axis=-1, keepdims=True)
    return idx.astype(np.int32), wts.astype(np.float32)


def _cap(n):
    return min(512, max(P, -(-n // 8) * 8))


def kernel(x, router_w, gate_w, up_w, down_w):
    import ml_dtypes

    bf = ml_dtypes.bfloat16

    x = np.asarray(x)
    in_dtype = x.dtype
    xf = x.reshape(-1, D).astype(np.float32)
    idx, wts = _route(x, np.asarray(router_w))

    # token lists per expert
    tok_ids = [None] * E
    tok_wts = [None] * E
    counts = np.zeros(E, dtype=np.int64)
    for e in range(E):
        sel = np.nonzero(idx == e)
        tok_ids[e] = sel[0].astype(np.int64)
        tok_wts[e] = wts[sel[0], sel[1]]
        counts[e] = len(tok_ids[e])

    # heaviest 8 experts -> slot 0 (capacity C0), lightest 8 -> slot 1 (C1)
    order = np.argsort(-counts, kind="stable")
    slot_exp = [(int(order[c]), int(order[8 + c])) for c in range(NCORES)]
    C0 = _cap(int(counts[order[0]]))
    C1 = _cap(int(counts[order[8]]))

    nc = _get_nc(C0, C1)

    KD, KF = D // P, F // P

    def tile_gateup(w):
        # [E, D, F] -> [E, KF, P, KD*P] with w_t[e,j,p,k*P+f] = w[e,k*P+p,j*P+f]
        w = np.asarray(w).astype(bf)
        w = w.reshape(E, KD, P, KF, P).transpose(0, 3, 2, 1, 4)
        return np.ascontiguousarray(w.reshape(E, KF, P, KD * P))

    g16 = tile_gateup(gate_w)
    u16 = tile_gateup(up_w)
    d16 = np.asarray(down_w).astype(bf)
    xT = np.ascontiguousarray(xf.T)  # [D, B*T] f32

    in_maps = []
    for c in range(NCORES):
        im = {}
        eids = slot_exp[c]
        for s, C in ((0, C0), (1, C1)):
            e = eids[s]
            n = int(counts[e])
            xg = np.zeros((P, KD, C), dtype=bf)
            gath = xT[:, tok_ids[e]]  # [D, n] f32
            xg[:, :, :n] = gath.astype(bf).reshape(KD, P, n).transpose(1, 0, 2)
            im[f"xgt{s}"] = xg.reshape(P, KD * C)
        im["wg"] = np.ascontiguousarray(g16[list(eids)])
        im["wu"] = np.ascontiguousarray(u16[list(eids)])
        im["wd"] = np.ascontiguousarray(d16[list(eids)])
        in_maps.append(im)

    res = run_bass_kernel_spmd(nc, in_maps, list(range(NCORES)))

    out = np.zeros((B * T, D), dtype=np.float32)
    for c in range(NCORES):
        for s in range(EPC):
            e = slot_exp[c][s]
            n = int(counts[e])
            yv = res.results[c][f"y{s}"]  # [D, C] bf16
            out[tok_ids[e]] += tok_wts[e][:, None] * yv[:, :n].astype(np.float32).T
    return out.reshape(B, T, D).astype(in_dtype)


# revision 29
# speedup vs baseline: 1.1366x; 1.1366x over previous
"""MoE layer (B=2,T=1024,D=2048,F=768,E=16,K=2) on 8 NeuronCores.

Expert-parallel: 16 experts sorted by routed-token count; the 8 heaviest go
in slot 0 (capacity C0), the 8 lightest in slot 1 (capacity C1 <= C0), one
of each per core. Host computes the router (softmax -> top-2 -> renormalize,
~0.3% of FLOPs), gathers each expert's tokens into fixed-capacity transposed
buffers, and the device kernel runs the sparse SwiGLU FFN in bf16 with f32
PSUM accumulation. The per-token routing weight is applied on the host
during the scatter-add, so no combine-weight tensor ships to the device.

The kernel sits at the ridge point: ~21MB of input at ~400GB/s aggregate
wire (~53us) vs ~68us of PE matmul issue. Hard-won scheduling facts:
- The two HWDGE rings (ACT=scalar, SP=sync) each sustain ~190GB/s when
  both are backlogged (HBM-wire split), FIFO per ring, so per-ring ORDER
  must equal deadline order and consumption must alternate rings:
  all gate halves ride the ACT ring, all up halves + tokens the SP ring.
- The HWDGE descriptor pool is SHARED and holds only a few transfers; a
  trigger issued while it is full BLOCKS the issuing engine for ~us.
  The sync engine is idle all kernel (its triggers may block freely, all
  queued up front), but the ACT engine runs silu + evictions, so only 4
  of its triggers are queued up front and the rest are popped one at a
  time after each silu / during pass A where ACT has slack.
- tokens staged transposed (xgt [D, C] as [P, KD*C]); e0's ship as 4
  quarter DMAs (separate tiles) so the j0 chain tracks the arrival ramp
  (g/u chains quarter-interleaved); a tiny warm-up DMA absorbs the SP
  ring's ~2us-slower first byte.
- e0 down projection runs in TWO full-PSUM passes (f-tiles 0-3, then
  4-5) with DVE-only copy evictions in pass A (ACT is kept free to
  absorb pop blocking) and ACT/DVE split in pass B; the partial merge
  (ysbA += ysbB) is deferred into e1's gate phase where the DVE is
  otherwise idle, and y(e0) streams out on the SWDGE queue behind it.
  This keeps the down passes PE-dense (a 3-pass staged variant was
  eviction-bound: DVE 86% busy) while the down-weight slab deadlines
  stay loose (dt00/dt01 by pass A, dt02 by pass B).
- e1's up halves stream on the SP ring as two paired transfers; e1's
  gate halves as three paired transfers popped from e0's last silu and
  pass A, ~9us ahead of their e1-gate deadlines (slow-wire cores eat
  2-3us of margin). e1 down is m-major with full 6-tile PSUM
  accumulation per m-chunk, so output DMAs spread across the phase; the
  last two m ship SOLO so the final post-matmul transfer is small.
- ~8 N=512 + 10 N=256 garbage matmuls at kernel start warm the PE clock
  (HAM) during the DMA ramp. The kernel-end tail is dominated by the
  fixed NEFF epilogue (~6.3us: a per-engine ladder resetting all 256
  semaphores, Tensor slowest at ~115ns each) plus ~2.5us of final
  y-drain + barrier; ~10.5us total is irreducible from the kernel side.
- fp8 (DoubleRow) was evaluated and rejected: weight-only e4m3 gives
  rel-err 5.1e-2 vs the 2e-2 gate (bf16: 5.5e-3), and with per-MM fresh
  weights DoubleRow is LDWEIGHTS-bound anyway (~1.2x at best).
Measured (quiet machine): ~92-95us max-core, ~91.5 mean-core, vs 97-98
for the previous baseline. NOTE: back-to-back runs trip a chip power
throttle (PE 2.4 -> ~2.0GHz, +15-20%); benchmark only after ~2min idle.
"""

import numpy as np
from contextlib import ExitStack

import concourse.bass as bass
import concourse.tile as tile
from concourse import mybir
from concourse.bass_utils import run_bass_kernel_spmd

B, T, D, F, E, TOPK = 2, 1024, 2048, 768, 16, 2
NCORES = 8
EPC = E // NCORES  # experts per core (2 slots)
P = 128


def _split_waits(nc, max_waits=1):
    """walrus on this image rejects >1 sync-wait per instruction
    (setupSyncWait: "Too many sync wait commands"); split extras into
    preceding same-engine NoOps."""
    for f in nc.m.functions:
        for b in f.blocks:
            insts = b.instructions
            idx = 0
            while idx < len(insts):
                inst = insts[idx]
                si = getattr(inst, "sync_info", None)
                if si is not None and si.on_wait and len(si.on_wait) > max_waits:
                    waits = list(si.on_wait)
                    extra, keep = waits[:-max_waits], waits[-max_waits:]
                    pos = idx
                    for j in range(0, len(extra), max_waits):
                        chunk = extra[j : j + max_waits]
                        nop = mybir.InstNoOp(name=f"{inst.name}_ws{j}", ins=[], outs=[])
                        nop.engine = inst.engine
                        nop.sync_info = mybir.SyncInfo(on_wait=chunk, on_update=[])
                        insts.insert(pos, nop)
                        pos += 1
                        idx += 1
                    inst.sync_info = mybir.SyncInfo(
                        on_wait=keep, on_update=list(si.on_update)
                    )
                idx += 1


def build_moe(C0, C1):
    """Per-core kernel: slot 0 capacity C0, slot 1 capacity C1 (each %8==0)."""
    assert C0 % 8 == 0 and C1 % 8 == 0 and 128 <= C1 <= C0 <= 512
    CS = (C0, C1)
    KD = D // P  # 16 k-tiles over D
    KF = F // P  # 6 f-chunks over F
    MD = D // P  # 16 m-chunks over D (down proj, yT layout)
    bf16 = mybir.dt.bfloat16
    f32 = mybir.dt.float32

    nc = bass.Bass("TRN2", target_bir_lowering=False, debug=False, num_devices=NCORES)
    # host pre-tiled layouts (>=2KB contiguous per partition per DMA):
    #   xgt{s}[p, k*C + c] = x_gathered[s, k*128+p, c]
    #   wgu[e, j, g, p, k*128+f] = w_g[e, k*128+p, j*128+f]  (g=0 gate, 1 up)
    xgt0 = nc.declare_dram_parameter("xgt0", [P, KD * C0], bf16, isOutput=False)
    xgt1 = nc.declare_dram_parameter("xgt1", [P, KD * C1], bf16, isOutput=False)
    wg = nc.declare_dram_parameter("wg", [EPC, KF, P, KD * P], bf16, isOutput=False)
    wu = nc.declare_dram_parameter("wu", [EPC, KF, P, KD * P], bf16, isOutput=False)
    wd = nc.declare_dram_parameter("wd", [EPC, F, D], bf16, isOutput=False)
    y0 = nc.declare_dram_parameter("y0", [D, C0], bf16, isOutput=True)
    y1 = nc.declare_dram_parameter("y1", [D, C1], bf16, isOutput=True)
    xgts = (xgt0, xgt1)
    ys = (y0, y1)

    with tile.TileContext(nc) as tc, ExitStack() as ctx:
        xp = ctx.enter_context(tc.tile_pool(name="xp", bufs=1))
        wgp = ctx.enter_context(tc.tile_pool(name="wgp", bufs=1))
        # dt tiles single-buffered: e1's down-weight DMA then naturally waits
        # for e0's pass reads to finish -- a free throttle that keeps that
        # traffic out of the oversubscribed early window.
        wdp = ctx.enter_context(tc.tile_pool(name="wdp", bufs=1))
        hp = ctx.enter_context(tc.tile_pool(name="hp", bufs=1))
        sp = ctx.enter_context(tc.tile_pool(name="sp", bufs=2))
        op = ctx.enter_context(tc.tile_pool(name="op", bufs=1))
        # one shared 8-bank PSUM ring.
        pp = ctx.enter_context(tc.tile_pool(name="pp", bufs=8, space="PSUM"))

        # PE warmup: garbage matmuls with no dependencies run during the
        # initial DMA ramp so HAM un-throttles (1.2->2.4GHz) before real work.
        wsb = sp.tile([P, 512], bf16, tag="warm_sb", bufs=1)
        nc.vector.memset(wsb[:], 0)
        for _ in range(8):
            wps = pp.tile([P, 512], f32, tag="ps")
            nc.tensor.matmul(wps[:], wsb[:, :P], wsb[:], start=True, stop=True)
        for _ in range(10):
            wps = pp.tile([P, 256], f32, tag="ps")
            nc.tensor.matmul(wps[:], wsb[:, :P], wsb[:, :256], start=True, stop=True)

        gts = [[None] * KF for _ in range(EPC)]  # (tile, idx) per j
        uts = [[None] * KF for _ in range(EPC)]
        dts = [[None] * 3 for _ in range(EPC)]
        xparts = [None, None]  # e0: 4 quarter tiles; e1: 1 full tile
        XKT = (4, KD)  # k-tiles per token tile

        def trig_w(e, j, eng, w, store, pref):
            t = wgp.tile(
                [P, 1, KD * P], bf16, tag=f"{pref}{e}j{j}", name=f"{pref}{e}j{j}",
                bufs=1,
            )
            eng.dma_start(t[:], w[e, j : j + 1].rearrange("j p c -> p j c"))
            store[e][j] = (t, 0)

        def trig_dt(e, h, eng):
            dt = wdp.tile([P, KF // 3, D], bf16, tag=f"dt{h}")
            eng.dma_start(
                dt[:],
                wd[e].rearrange("(k p) d -> p k d", p=P)[:, bass.ts(h, KF // 3), :],
            )
            dts[e][h] = dt

        def xop(e, k):
            """Column-slice covering token k-tile k of expert e."""
            kt = XKT[e]
            return xparts[e][k // kt][:, bass.ts(k % kt, CS[e])]

        # Two HWDGE rings, each ~190GB/s when both are backlogged (the
        # aggregate is HBM-wire-bound), FIFO per ring.  The gate phase
        # consumes ~265GB/s, so consumption must ALTERNATE rings every
        # half-slab: all gate halves ride the ACT ring, all up halves and
        # tokens the SP ring, each in deadline order -- per-ring demand is
        # then ~134GB/s and the j-loop stays PE-bound after j1.  A tiny
        # warm-up DMA absorbs the SP ring's ~2us-slower first byte.  All
        # triggers are issued up front (the ACT engine finishes its 15
        # before its first silu is due; the sync engine is idle anyway).
        wdm = sp.tile([P, 16], bf16, tag="warm_dma", bufs=1)
        nc.sync.dma_start(wdm[:], xgt0[:, :16])

        # The HWDGE ring backpressures its trigger instructions once ~4
        # transfers are in flight, so an engine that has real work (ACT:
        # silu + evictions) may only queue 4 up front -- the rest are
        # drip-fed from issue points where the ring has drained (after
        # each silu / between pass-A evictions).  The sync engine is idle
        # all kernel, so its ring's triggers can all block harmlessly up
        # front.
        trig_w(0, 0, nc.scalar, wg, gts, "g")
        # e0 tokens: 4 quarter DMAs (separate tiles so each quarter's
        # consumers wait only for its own transfer).
        xparts[0] = []
        for q in range(4):
            xt = xp.tile([P, 4 * C0], bf16, tag=f"xt0q{q}")
            nc.sync.dma_start(xt[:], xgt0[:, bass.ts(q, 4 * C0)])
            xparts[0].append(xt)
        trig_w(0, 0, nc.scalar, wu, uts, "u")
        trig_w(0, 1, nc.scalar, wg, gts, "g")
        trig_w(0, 2, nc.scalar, wg, gts, "g")
        for j in range(1, KF):
            trig_w(0, j, nc.sync, wu, uts, "u")
        trig_dt(0, 1, nc.sync)
        # e1 tokens (one fused DMA) + up halves (two fused transfers)
        # behind on the SP ring.
        xt1 = xp.tile([P, KD * C1], bf16, tag="xt1")
        nc.sync.dma_start(xt1[:], xgt1[:, :])
        xparts[1] = [xt1]

        def trig_w3(e, j0, eng, w, store, pref):
            t = wgp.tile(
                [P, 3, KD * P], bf16, tag=f"{pref}{e}g{j0}", name=f"{pref}{e}g{j0}",
                bufs=1,
            )
            eng.dma_start(t[:], w[e, j0 : j0 + 3].rearrange("j p c -> p j c"))
            for i in range(3):
                store[e][j0 + i] = (t, i)

        trig_w3(1, 0, nc.sync, wu, uts, "u")
        trig_w3(1, 3, nc.sync, wu, uts, "u")

        def trig_w2(e, j0, eng, w, store, pref):
            t = wgp.tile(
                [P, 2, KD * P], bf16, tag=f"{pref}{e}p{j0}", name=f"{pref}{e}p{j0}",
                bufs=1,
            )
            eng.dma_start(t[:], w[e, j0 : j0 + 2].rearrange("j p c -> p j c"))
            for i in range(2):
                store[e][j0 + i] = (t, i)

        # ACT-ring triggers deferred past the 4 upfront slots, in ring
        # (deadline) order; popped one at a time after each silu while the
        # ACT engine has slack.  e1's gate halves ship as three paired
        # transfers, the first popped at e0's last silu and the rest
        # during pass A, where the ACT engine does nothing else (pass A
        # evictions are DVE-only) so the pops can absorb descriptor-pool
        # blocking harmlessly while still arriving ~9us before their
        # e1-gate deadlines.
        act_trigs = (
            [(lambda j=j: trig_w(0, j, nc.scalar, wg, gts, "g")) for j in (3, 4)]
            + [lambda: trig_dt(0, 0, nc.scalar)]
            + [lambda: trig_w(0, 5, nc.scalar, wg, gts, "g")]
            + [lambda: trig_dt(0, 2, nc.scalar)]
            + [None, lambda: trig_w2(1, 0, nc.scalar, wg, gts, "g")]
        )
        e1g_trigs = [
            (lambda j0=j0: trig_w2(1, j0, nc.scalar, wg, gts, "g")) for j0 in (2, 4)
        ]

        def pop_trig(n=1, src=act_trigs):
            for _ in range(n):
                if src:
                    t = src.pop(0)
                    if t is not None:
                        t()

        for e in range(EPC):
            C = CS[e]
            # ---- gate/up + SwiGLU -> hT [F, C] bf16 ----
            ht = hp.tile([P, KF, C], bf16, tag=f"ht{e}")
            for j in range(KF):
                gt, gi = gts[e][j]
                ut, ui = uts[e][j]
                g_ps = pp.tile([P, C], f32, tag="ps")
                u_ps = pp.tile([P, C], f32, tag="ps")
                if e == 0 and j == 0:
                    # bootstrap: interleave the g/u chains by token
                    # quarter so the PE tracks the arrival ramp instead
                    # of stalling per quarter twice.
                    order = [("g", k) for k in range(4)] + [("u", k) for k in range(4)]
                    order = [
                        ("g", 0), ("g", 1), ("g", 2), ("u", 0),
                        ("u", 1), ("g", 3), ("u", 2), ("u", 3),
                    ]
                    for which, q in order:
                        ps, (t, ti) = (
                            (g_ps, (gt, gi)) if which == "g" else (u_ps, (ut, ui))
                        )
                        for k in range(4 * q, 4 * q + 4):
                            nc.tensor.matmul(
                                ps[:],
                                t[:, ti, bass.ts(k, P)],
                                xop(e, k),
                                start=(k == 0),
                                stop=(k == KD - 1),
                            )
                else:
                    for k in range(KD):
                        nc.tensor.matmul(
                            g_ps[:],
                            gt[:, gi, bass.ts(k, P)],
                            xop(e, k),
                            start=(k == 0),
                            stop=(k == KD - 1),
                        )
                    for k in range(KD):
                        nc.tensor.matmul(
                            u_ps[:],
                            ut[:, ui, bass.ts(k, P)],
                            xop(e, k),
                            start=(k == 0),
                            stop=(k == KD - 1),
                        )
                sil = sp.tile([P, C], f32, tag="sil")
                nc.scalar.activation(
                    sil[:], g_ps[:], mybir.ActivationFunctionType.Silu
                )
                if e == 0:
                    pop_trig(2 if j == 0 else 1)
                nc.vector.tensor_mul(ht[:, j, :], sil[:], u_ps[:])
                if e == 1:
                    # deferred e0 partial merge + y0 drain ride the
                    # otherwise-idle DVE / SWDGE queue during e1's gate.
                    if j < 4:
                        m0 = 4 * j
                        nc.vector.tensor_add(
                            ysbA[:, m0 : m0 + 4, :],
                            ysbA[:, m0 : m0 + 4, :],
                            ysbB[:, m0 : m0 + 4, :],
                        )
                        nc.gpsimd.dma_start(
                            ys[0].rearrange("(m p) c -> p m c", p=P)[
                                :, m0 : m0 + 4, :
                            ],
                            ysbA[:, m0 : m0 + 4, :],
                        )

            # ---- down proj: yT[m] = sum_f dwT[f, m] @ hT[f, :] ----
            ydst = ys[e].rearrange("(m p) c -> p m c", p=P)
            if e == 0:
                # two full-PSUM passes (f-tiles 0-3, then 4-5): copy-only
                # evictions split ACT/DVE by m-parity keep the PE dense;
                # the A+B merge is deferred into e1's gate phase.
                ysbA = op.tile([P, MD, C], bf16, tag="ysbA")
                ysbB = op.tile([P, MD, C], bf16, tag="ysbB")
                for m in range(MD):
                    y_ps = pp.tile([P, C], f32, tag="ps")
                    for hi in range(4):
                        nc.tensor.matmul(
                            y_ps[:],
                            dts[0][hi // 2][:, hi % 2, bass.ts(m, P)],
                            ht[:, hi, :],
                            start=(hi == 0),
                            stop=(hi == 3),
                        )
                    # DVE-only evictions: the ACT engine stays free so the
                    # two e1 gate-weight pops can block on descriptor-pool
                    # space without stalling anything.
                    nc.vector.tensor_copy(ysbA[:, m, :], y_ps[:])
                    if m in (2, 6):
                        pop_trig(1, e1g_trigs)
                # e1 down slabs join the queues here; WAR-gated on e0's
                # pass reads (wdp bufs=1), and their ring backlog places
                # them well before e1's down phase.
                trig_dt(1, 0, nc.scalar)
                trig_dt(1, 1, nc.sync)
                trig_dt(1, 2, nc.sync)
                for m in range(MD):
                    y_ps = pp.tile([P, C], f32, tag="ps")
                    for i in range(2):
                        nc.tensor.matmul(
                            y_ps[:],
                            dts[0][2][:, i, bass.ts(m, P)],
                            ht[:, 4 + i, :],
                            start=(i == 0),
                            stop=(i == 1),
                        )
                    ev = nc.vector.tensor_copy if m % 2 else nc.scalar.copy
                    ev(ysbB[:, m, :], y_ps[:])
                    if m % 8 == 3:
                        pop_trig(1, e1g_trigs)
            else:
                # input is all on-chip: m-major, full 6-tile accumulation
                # in PSUM per m-chunk, one eviction per m-chunk (split
                # ACT/DVE by parity).  m-major means each m completes
                # early in the phase, so the output DMAs spread across it
                # instead of bunching at the end; the last two m ship as
                # SOLO transfers so the final post-matmul DMA is one small
                # transfer on an empty ring (the kernel-end barrier waits
                # on its receipt).
                ysb = op.tile([P, MD, C], bf16, tag="ysb1")
                for m in range(MD):
                    y_ps = pp.tile([P, C], f32, tag="ps")
                    for hi in range(6):
                        nc.tensor.matmul(
                            y_ps[:],
                            dts[e][hi // 2][:, hi % 2, bass.ts(m, P)],
                            ht[:, hi, :],
                            start=(hi == 0),
                            stop=(hi == 5),
                        )
                    ev = nc.vector.tensor_copy if m % 2 else nc.scalar.copy
                    ev(ysb[:, m, :], y_ps[:])
                    if m < 14 and m % 2 == 1:
                        yeng = nc.sync if (m // 2) % 2 == 0 else nc.scalar
                        yeng.dma_start(
                            ydst[:, m - 1 : m + 1, :],
                            ysb[:, m - 1 : m + 1, :],
                        )
                    elif m == 14:
                        nc.scalar.dma_start(
                            ydst[:, m : m + 1, :], ysb[:, m : m + 1, :]
                        )
                    elif m == 15:
                        nc.sync.dma_start(
                            ydst[:, m : m + 1, :], ysb[:, m : m + 1, :]
                        )

    _split_waits(nc)
    return nc


_CACHE = {}


def _get_nc(C0, C1):
    if (C0, C1) not in _CACHE:
        _CACHE[(C0, C1)] = build_moe(C0, C1)
    return _CACHE[(C0, C1)]


def _route(x, router_w):
    """Replicates the reference router in f32: softmax over expert scores,
    top-2, renormalize."""
    xf = x.reshape(-1, D).astype(np.float32)
    scores = xf @ router_w.astype(np.float32)
    m = scores.max(axis=-1, keepdims=True)
    ex = np.exp(scores - m)
    probs = ex / ex.sum(axis=-1, keepdims=True)
    idx = np.argsort(-probs, axis=-1, kind="stable")[:, :TOPK]
    wts = np.take_along_axis(probs, idx, axis=-1)
    wts = wts / wts.sum(axis=-1, keepdims=True)
    return idx.astype(np.int32), wts.astype(np.float32)


def _cap(n):
    return min(512, max(P, -(-n // 8) * 8))


def kernel(x, router_w, gate_w, up_w, down_w):
    import ml_dtypes

    bf = ml_dtypes.bfloat16

    x = np.asarray(x)
    in_dtype = x.dtype
    xf = x.reshape(-1, D).astype(np.float32)
    idx, wts = _route(x, np.asarray(router_w))

    # token lists per expert
    tok_ids = [None] * E
    tok_wts = [None] * E
    counts = np.zeros(E, dtype=np.int64)
    for e in range(E):
        sel = np.nonzero(idx == e)
        tok_ids[e] = sel[0].astype(np.int64)
        tok_wts[e] = wts[sel[0], sel[1]]
        counts[e] = len(tok_ids[e])

    # heaviest 8 experts -> slot 0 (capacity C0), lightest 8 -> slot 1 (C1)
    order = np.argsort(-counts, kind="stable")
    slot_exp = [(int(order[c]), int(order[8 + c])) for c in range(NCORES)]
    C0 = _cap(int(counts[order[0]]))
    C1 = _cap(int(counts[order[8]]))

    nc = _get_nc(C0, C1)

    KD, KF = D // P, F // P

    def tile_gateup(w):
        # [E, D, F] -> [E, KF, P, KD*P] with w_t[e,j,p,k*P+f] = w[e,k*P+p,j*P+f]
        w = np.asarray(w).astype(bf)
        w = w.reshape(E, KD, P, KF, P).transpose(0, 3, 2, 1, 4)
        return np.ascontiguousarray(w.reshape(E, KF, P, KD * P))

    g16 = tile_gateup(gate_w)
    u16 = tile_gateup(up_w)
    d16 = np.asarray(down_w).astype(bf)
    xT = np.ascontiguousarray(xf.T)  # [D, B*T] f32

    in_maps = []
    for c in range(NCORES):
        im = {}
        eids = slot_exp[c]
        for s, C in ((0, C0), (1, C1)):
            e = eids[s]
            n = int(counts[e])
            xg = np.zeros((P, KD, C), dtype=bf)
            gath = xT[:, tok_ids[e]]  # [D, n] f32
            xg[:, :, :n] = gath.astype(bf).reshape(KD, P, n).transpose(1, 0, 2)
            im[f"xgt{s}"] = xg.reshape(P, KD * C)
        im["wg"] = np.ascontiguousarray(g16[list(eids)])
        im["wu"] = np.ascontiguousarray(u16[list(eids)])
        im["wd"] = np.ascontiguousarray(d16[list(eids)])
        in_maps.append(im)

    res = run_bass_kernel_spmd(nc, in_maps, list(range(NCORES)))

    out = np.zeros((B * T, D), dtype=np.float32)
    for c in range(NCORES):
        for s in range(EPC):
            e = slot_exp[c][s]
            n = int(counts[e])
            yv = res.results[c][f"y{s}"]  # [D, C] bf16
            out[tok_ids[e]] += tok_wts[e][:, None] * yv[:, :n].astype(np.float32).T
    return out.reshape(B, T, D).astype(in_dtype)
